# revision 18
# baseline (speedup 1.0000x reference)
"""Trainium2 Bass kernel for BertSelfAttention with C_prior multiply.

Reference (per batch b):
  q/k/v = x @ W{q,k,v}.T + b{q,k,v}            -> [S, D], split into H=16 heads of W=64
  scores = q k^T / sqrt(W); mask; softmax over k
  attn = softmax(scores) * C_prior[b]
  out = attn @ v                               -> [B, S, D]

Shapes: B=2, S=2048, D=1024, H=16, W=64.

Sharding: 8 cores; core c owns batch b=c//4 and 4 consecutive heads
(hg=c%4 -> heads 4*hg..4*hg+3). The whole per-(b,h) score block stays local.

Device layout strategy (per core):
  - Host pre-transposes inputs so the device never transposes anything big:
      xT = x[b].T                                  [D, S]
      wqk = per-pair [Wq_h0^T|Wq_h1^T|Wk_h0^T|Wk_h1^T] column blocks
      ct  = (C_prior[b].T * mask) as bf16          [S, S]   (k-major)
  - Projections produce Q^T/K^T in [w, q] layout and V in natural [k, w].
  - scoresT = K Q^T computed directly in [k, q] layout (matmul lhsT=K^T,
    rhs=Q^T), so softmax's exp output feeds the A@V matmul with k already
    on partitions -- no on-chip transpose of the big attention matrix.
  - softmax denominator: ones(mask)-vector matmul over expS^T partitions,
    accumulated in PSUM across k-strips; exp skips max-subtraction
    (scores ~ N(0,1), no overflow risk in fp32).
  - attn*C: single VE bf16 tensor_tensor multiply per strip.
  - Output O^T [w, q] is written per head; the host transposes the small
    result during the gather/unshard step.

Matmuls run as float32r (full PE rate) for the fp32 path; the attention
matrix path (expS^T, C^T, V) runs bf16.
"""

import os

import numpy as np
import ml_dtypes

B, S, D, H, W = 2, 2048, 1024, 16, 64
NCORES = 8
HEADS_PER_CORE = 4
P = 128
QH = S // 2  # q processed in two halves of 1024 to fit PSUM

_prog_cache = {}


def _build_program():
    import concourse.mybir as mybir
    import concourse.tile as tile
    from concourse import bacc

    dt = mybir.dt
    f32, bf16 = dt.float32, dt.bfloat16
    Alu = mybir.AluOpType
    Act = mybir.ActivationFunctionType

    nc = bacc.Bacc("TRN2", target_bir_lowering=False)

    xT_d = nc.declare_dram_parameter("xT", [D, S], bf16, isOutput=False)
    wqk_d = nc.declare_dram_parameter("wqk", [D, 512], bf16, isOutput=False)
    wv_d = nc.declare_dram_parameter("wv", [D, 256], bf16, isOutput=False)
    bqk_d = nc.declare_dram_parameter("bqk", [P, 4], f32, isOutput=False)
    bvr_d = nc.declare_dram_parameter("bvr", [P, 256], f32, isOutput=False)
    ct_d = nc.declare_dram_parameter("ct", [S, S], bf16, isOutput=False)
    mk_d = nc.declare_dram_parameter("mk", [P, 16 * 64], bf16, isOutput=False)
    out_d = nc.declare_dram_parameter("out", [256, S], f32, isOutput=True)

    NK = S // P  # 16 k-strips
    NPH = 4  # phases: (qh, pair)
    BOFF = 8  # pass B trails pass A by 8 strips

    with tile.TileContext(nc) as tc:
        with tc.tile_pool(name="persist", bufs=1) as persist:
            qk_all = persist.tile([P, 4, S], bf16)
            v_sb = persist.tile([P, NK, 256], bf16)
            bqk_sb = persist.tile([P, 4], f32)
            bvr_sb = persist.tile([P, 256], f32)
            mk_sb = persist.tile([P, NK, 64], bf16)
            xT_sb = persist.tile([P, 8, S], bf16)
            wqk_sb = persist.tile([P, 8, 512], bf16)
            wv_sb = persist.tile([P, 8, 256], bf16)
            nc.sync.dma_start(out=bqk_sb[:], in_=bqk_d[:])
            nc.sync.dma_start(out=bvr_sb[:], in_=bvr_d[:])
            nc.sync.dma_start(out=mk_sb[:], in_=mk_d[:])
            xT_r = xT_d.rearrange("(o p) q -> p o q", p=P)
            wqk_r = wqk_d.rearrange("(o p) m -> p o m", p=P)
            wv_r = wv_d.rearrange("(o p) m -> p o m", p=P)
            for dc in range(8):
                nc.sync.dma_start(out=wqk_sb[:, dc, :], in_=wqk_r[:, dc, :])
                nc.sync.dma_start(out=xT_sb[:, dc, :], in_=xT_r[:, dc, :])
                nc.sync.dma_start(out=wv_sb[:, dc, :], in_=wv_r[:, dc, :])

            with tc.tile_pool(name="estr", bufs=22) as ep, tc.tile_pool(name="astr", bufs=4) as app, tc.tile_pool(
                name="ctp", bufs=4
            ) as ctp, tc.tile_pool(name="small", bufs=1) as smallp, tc.tile_pool(
                name="mm1ps", bufs=2, space="PSUM"
            ) as mm1p, tc.tile_pool(
                name="ops", bufs=1, space="PSUM"
            ) as pop, tc.tile_pool(
                name="rsps", bufs=1, space="PSUM"
            ) as prsp:

                def proj_qk(col, qb):
                    ps = mm1p.tile([P, 512], f32, tag="scT")
                    for dc in range(8):
                        nc.tensor.matmul(
                            ps[:],
                            lhsT=wqk_sb[:, dc, col * P : (col + 1) * P],
                            rhs=xT_sb[:, dc, qb * 512 : (qb + 1) * 512],
                            start=(dc == 0),
                            stop=(dc == 7),
                        )
                    nc.vector.tensor_scalar_add(
                        out=qk_all[:, col, qb * 512 : (qb + 1) * 512],
                        in0=ps[:],
                        scalar1=bqk_sb[:, col : col + 1],
                    )

                def proj_v(kt):
                    ps = mm1p.tile([P, 256], f32, tag="scT")
                    for dc in range(8):
                        nc.tensor.matmul(
                            ps[:],
                            lhsT=xT_sb[:, dc, kt * P : (kt + 1) * P],
                            rhs=wv_sb[:, dc, :],
                            start=(dc == 0),
                            stop=(dc == 7),
                        )
                    nc.vector.tensor_tensor(v_sb[:, kt, :], ps[:], bvr_sb[:], Alu.add)

                def passA_strip(qh, pr, ks):
                    psA = mm1p.tile([P, QH], f32, tag="scT")
                    psB = mm1p.tile([P, QH], f32, tag="scT")
                    for q2 in range(2):
                        qs = slice(qh * QH + q2 * 512, qh * QH + (q2 + 1) * 512)
                        os_ = slice(q2 * 512, (q2 + 1) * 512)
                        nc.tensor.matmul(
                            psA[:, os_],
                            lhsT=qk_all[0:64, 2 * pr + 1, ks * P : (ks + 1) * P],
                            rhs=qk_all[0:64, 2 * pr, qs],
                            tile_position=(0, 0),
                            start=True,
                            stop=True,
                        )
                        nc.tensor.matmul(
                            psB[:, os_],
                            lhsT=qk_all[64:128, 2 * pr + 1, ks * P : (ks + 1) * P],
                            rhs=qk_all[64:128, 2 * pr, qs],
                            tile_position=(64, 0),
                            start=True,
                            stop=True,
                        )
                    eA = ep.tile([P, QH], bf16, tag="e")
                    nc.scalar.activation(eA[:], psA[:], Act.Exp, scale=0.125)
                    eB = ep.tile([P, QH], bf16, tag="e")
                    nc.scalar.activation(eB[:], psB[:], Act.Exp, scale=0.125)
                    return eA, eB

                def passB_strip(qh, pr, ks, eA, eB, po, prs):
                    h0, h1 = 2 * pr, 2 * pr + 1
                    ct = ctp.tile([P, QH], bf16, tag="ct")
                    nc.sync.dma_start(
                        out=ct[:],
                        in_=ct_d[ks * P : (ks + 1) * P, qh * QH : (qh + 1) * QH],
                    )
                    aA = app.tile([P, QH], bf16, tag="a")
                    nc.vector.tensor_tensor(aA[:], eA[:], ct[:], Alu.mult)
                    aB = app.tile([P, QH], bf16, tag="a")
                    nc.vector.tensor_tensor(aB[:], eB[:], ct[:], Alu.mult)
                    st, sp = (ks == 0), (ks == NK - 1)
                    for q2 in range(2):
                        os_ = slice(q2 * 512, (q2 + 1) * 512)
                        nc.tensor.matmul(
                            prs[0:64, os_],
                            lhsT=mk_sb[:, ks, :],
                            rhs=eA[:, os_],
                            tile_position=(0, 0),
                            start=st,
                            stop=sp,
                        )
                        nc.tensor.matmul(
                            prs[64:128, os_],
                            lhsT=mk_sb[:, ks, :],
                            rhs=eB[:, os_],
                            tile_position=(0, 64),
                            start=st,
                            stop=sp,
                        )
                        nc.tensor.matmul(
                            po[0:64, os_],
                            lhsT=v_sb[:, ks, h0 * 64 : (h0 + 1) * 64],
                            rhs=aA[:, os_],
                            tile_position=(0, 0),
                            start=st,
                            stop=sp,
                        )
                        nc.tensor.matmul(
                            po[64:128, os_],
                            lhsT=v_sb[:, ks, h1 * 64 : (h1 + 1) * 64],
                            rhs=aB[:, os_],
                            tile_position=(0, 64),
                            start=st,
                            stop=sp,
                        )

                def finishB(qh, pr, po, prs):
                    rcs = smallp.tile([P, QH], f32, tag="rcs")
                    scr = smallp.tile([P, QH], f32, tag="scr")
                    nc.vector.reciprocal_approx_accurate(rcs[:], prs[:], scr[:])
                    ob = smallp.tile([P, QH], f32, tag="ob")
                    nc.vector.tensor_tensor(ob[:], po[:], rcs[:], Alu.mult)
                    nc.sync.dma_start(
                        out=out_d[pr * P : (pr + 1) * P, qh * QH : (qh + 1) * QH],
                        in_=ob[:],
                    )

                phases = [(qh, pr) for qh in range(2) for pr in range(2)]
                # pair-0 Q/K up front; pair-1 Q/K + all of V are deferred into
                # phase-0 iterations (no pass B there yet).
                for qb in range(4):
                    proj_qk(0, qb)
                    proj_qk(1, qb)

                es_all = {}
                bstate = {}
                NITER = NPH * NK + BOFF
                for g in range(NITER):
                    ph, ks = g // NK, g % NK
                    if g < NPH * NK:
                        qh, pr = phases[ph]
                        es_all.setdefault(ph, []).append(passA_strip(qh, pr, ks))
                        if ph == 0:
                            proj_v(ks)
                            if ks < 8:
                                proj_qk(2 + ks // 4, ks % 4)
                    bg = g - BOFF
                    if bg >= 0:
                        bph, bks = bg // NK, bg % NK
                        bqh, bpr = phases[bph]
                        if bks == 0:
                            b_po = pop.tile([P, QH], f32, tag="po", name="po")
                            b_prs = prsp.tile(
                                [P, QH], f32, tag="prs", name="prs"
                            )
                            bstate[bph] = (b_po, b_prs)
                        b_po, b_prs = bstate[bph]
                        passB_strip(bqh, bpr, bks, *es_all[bph][bks], b_po, b_prs)
                        if bks == NK - 1:
                            finishB(bqh, bpr, b_po, b_prs)
                            del bstate[bph]
                            del es_all[bph]

    nc.finalize()
    return nc


def _get_program():
    if "nc" not in _prog_cache:
        _prog_cache["nc"] = _build_program()
    return _prog_cache["nc"]


def kernel(x, attention_mask, C_prior, Wq, bq, Wk, bk, Wv, bv):
    from concourse.bass_utils import run_bass_kernel_spmd

    x = np.asarray(x, dtype=np.float32)
    attention_mask = np.asarray(attention_mask)
    C_prior = np.asarray(C_prior, dtype=np.float32)
    Wq = np.asarray(Wq, dtype=np.float32)
    Wk = np.asarray(Wk, dtype=np.float32)
    Wv = np.asarray(Wv, dtype=np.float32)
    bq = np.asarray(bq, dtype=np.float32)
    bk = np.asarray(bk, dtype=np.float32)
    bv = np.asarray(bv, dtype=np.float32)
    bf = ml_dtypes.bfloat16

    WqT, WkT, WvT = Wq.T, Wk.T, Wv.T  # [in D, out D]
    maskf = attention_mask.astype(np.float32)  # [B, S]

    in_maps = []
    for c in range(NCORES):
        b, hg = c // 4, c % 4
        heads = [4 * hg + i for i in range(HEADS_PER_CORE)]
        xT = np.ascontiguousarray(x[b].T).astype(bf)  # [D, S]

        wqk = np.empty((D, 512), np.float32)
        bqk = np.zeros((P, 4), np.float32)
        for pr in range(2):
            h0, h1 = heads[2 * pr], heads[2 * pr + 1]
            wqk[:, (2 * pr) * P : (2 * pr) * P + 64] = WqT[:, h0 * 64 : h0 * 64 + 64]
            wqk[:, (2 * pr) * P + 64 : (2 * pr + 1) * P] = WqT[
                :, h1 * 64 : h1 * 64 + 64
            ]
            wqk[:, (2 * pr + 1) * P : (2 * pr + 1) * P + 64] = WkT[
                :, h0 * 64 : h0 * 64 + 64
            ]
            wqk[:, (2 * pr + 1) * P + 64 : (2 * pr + 2) * P] = WkT[
                :, h1 * 64 : h1 * 64 + 64
            ]
            bqk[0:64, 2 * pr] = bq[h0 * 64 : h0 * 64 + 64]
            bqk[64:128, 2 * pr] = bq[h1 * 64 : h1 * 64 + 64]
            bqk[0:64, 2 * pr + 1] = bk[h0 * 64 : h0 * 64 + 64]
            bqk[64:128, 2 * pr + 1] = bk[h1 * 64 : h1 * 64 + 64]

        wv = np.ascontiguousarray(WvT[:, heads[0] * 64 : (heads[-1] + 1) * 64]).astype(bf)
        bvr = np.ascontiguousarray(
            np.broadcast_to(
                bv[heads[0] * 64 : (heads[-1] + 1) * 64][None, :], (P, 256)
            )
        )
        m = maskf[b]  # [S]
        ct = (C_prior[b].T * m[:, None]).astype(bf)  # [S(k), S(q)] * mask[k]
        mkcol = m.reshape(S // P, P).T.astype(bf)  # [P, 16]
        mk = np.ascontiguousarray(
            np.repeat(mkcol[:, :, None], 64, axis=2).reshape(P, -1)
        )  # [P, 16*64]

        in_maps.append(
            {
                "xT": xT,
                "wqk": wqk.astype(bf),
                "wv": wv,
                "bqk": bqk,
                "bvr": bvr,
                "ct": ct,
                "mk": mk,
            }
        )

    nc = _get_program()
    trace = bool(int(os.environ.get("BASS_KERNEL_TRACE", "0")))
    res = run_bass_kernel_spmd(nc, in_maps, list(range(NCORES)), trace=trace)
    if trace:
        print(f"HW exec time: {res.exec_time_ns} ns")
        _prog_cache["last_exec_time_ns"] = res.exec_time_ns
        _prog_cache["last_trace"] = res.instructions_and_trace

    out = np.empty((B, S, D), np.float32)
    for c in range(NCORES):
        b, hg = c // 4, c % 4
        co = res.results[c]["out"]  # [256, S]
        for i in range(HEADS_PER_CORE):
            h = 4 * hg + i
            out[b, :, h * 64 : (h + 1) * 64] = co[i * 64 : (i + 1) * 64, :].T
    return out


# revision 19
# speedup vs baseline: 1.0758x; 1.0758x over previous
"""Trainium2 Bass kernel for BertSelfAttention with C_prior multiply.

Reference (per batch b):
  q/k/v = x @ W{q,k,v}.T + b{q,k,v}            -> [S, D], split into H=16 heads of W=64
  scores = q k^T / sqrt(W); mask; softmax over k
  attn = softmax(scores) * C_prior[b]
  out = attn @ v                               -> [B, S, D]

Shapes: B=2, S=2048, D=1024, H=16, W=64.

Sharding: 8 cores; core c owns batch b=c//4 and 4 consecutive heads
(hg=c%4 -> heads 4*hg..4*hg+3). The whole per-(b,h) score block stays local.

Device layout strategy (per core):
  - Host pre-transposes inputs so the device never transposes anything big:
      xT = x[b].T                                  [D, S]
      wqk = per-pair [Wq_h0^T|Wq_h1^T|Wk_h0^T|Wk_h1^T] column blocks
      ct  = (C_prior[b].T * mask) as bf16          [S, S]   (k-major)
  - Projections produce Q^T/K^T in [w, q] layout and V in natural [k, w].
  - scoresT = K Q^T computed directly in [k, q] layout (matmul lhsT=K^T,
    rhs=Q^T), so softmax's exp output feeds the A@V matmul with k already
    on partitions -- no on-chip transpose of the big attention matrix.
  - softmax denominator: ones(mask)-vector matmul over expS^T partitions,
    accumulated in PSUM across k-strips; exp skips max-subtraction
    (scores ~ N(0,1), no overflow risk in fp32).
  - attn*C: single VE bf16 tensor_tensor multiply per strip.
  - Output O^T [w, q] is written per head; the host transposes the small
    result during the gather/unshard step.

Matmuls run as float32r (full PE rate) for the fp32 path; the attention
matrix path (expS^T, C^T, V) runs bf16.
"""

import os

import numpy as np
import ml_dtypes

B, S, D, H, W = 2, 2048, 1024, 16, 64
NCORES = 8
HEADS_PER_CORE = 4
P = 128
QH = S // 2  # q processed in two halves of 1024 to fit PSUM

_prog_cache = {}


def _build_program():
    import concourse.mybir as mybir
    import concourse.tile as tile
    from concourse import bacc

    dt = mybir.dt
    f32, bf16 = dt.float32, dt.bfloat16
    Alu = mybir.AluOpType
    Act = mybir.ActivationFunctionType

    nc = bacc.Bacc("TRN2", target_bir_lowering=False)

    xT_d = nc.declare_dram_parameter("xT", [D, S], bf16, isOutput=False)
    wqk_d = nc.declare_dram_parameter("wqk", [D, 512], bf16, isOutput=False)
    wv_d = nc.declare_dram_parameter("wv", [D, 256], bf16, isOutput=False)
    bqk_d = nc.declare_dram_parameter("bqk", [P, 4], f32, isOutput=False)
    bvr_d = nc.declare_dram_parameter("bvr", [P, 256], f32, isOutput=False)
    ct_d = nc.declare_dram_parameter("ct", [S, S], bf16, isOutput=False)
    mk_d = nc.declare_dram_parameter("mk", [P, 16 * 64], bf16, isOutput=False)
    out_d = nc.declare_dram_parameter("out", [256, S], f32, isOutput=True)

    NK = S // P  # 16 k-strips
    NPH = 4  # phases: (qh, pair)
    BOFF = 8  # pass B trails pass A by 8 strips

    with tile.TileContext(nc) as tc:
        with tc.tile_pool(name="persist", bufs=1) as persist:
            qk_all = persist.tile([P, 4, S], bf16)
            v_sb = persist.tile([P, NK, 256], bf16)
            bqk_sb = persist.tile([P, 4], f32)
            bvr_sb = persist.tile([P, 256], f32)
            mk_sb = persist.tile([P, NK, 64], bf16)
            xT_sb = persist.tile([P, 8, S], bf16)
            wqk_sb = persist.tile([P, 8, 512], bf16)
            wv_sb = persist.tile([P, 8, 256], bf16)
            nc.sync.dma_start(out=bqk_sb[:], in_=bqk_d[:])
            nc.sync.dma_start(out=bvr_sb[:], in_=bvr_d[:])
            nc.sync.dma_start(out=mk_sb[:], in_=mk_d[:])
            xT_r = xT_d.rearrange("(o p) q -> p o q", p=P)
            wqk_r = wqk_d.rearrange("(o p) m -> p o m", p=P)
            wv_r = wv_d.rearrange("(o p) m -> p o m", p=P)
            for dc in range(8):
                nc.sync.dma_start(out=wqk_sb[:, dc, :], in_=wqk_r[:, dc, :])
                nc.sync.dma_start(out=xT_sb[:, dc, :], in_=xT_r[:, dc, :])
                nc.sync.dma_start(out=wv_sb[:, dc, :], in_=wv_r[:, dc, :])

            with tc.tile_pool(name="estr", bufs=22) as ep, tc.tile_pool(name="astr", bufs=4) as app, tc.tile_pool(
                name="ctp", bufs=4
            ) as ctp, tc.tile_pool(name="small", bufs=1) as smallp, tc.tile_pool(
                name="mm1ps", bufs=4, space="PSUM"
            ) as mm1p, tc.tile_pool(
                name="ops", bufs=1, space="PSUM"
            ) as pop, tc.tile_pool(
                name="rsps", bufs=1, space="PSUM"
            ) as prsp:

                def proj_qk(col, qb):
                    ps = mm1p.tile([P, 512], f32, tag="scT")
                    for dc in range(8):
                        nc.tensor.matmul(
                            ps[:],
                            lhsT=wqk_sb[:, dc, col * P : (col + 1) * P],
                            rhs=xT_sb[:, dc, qb * 512 : (qb + 1) * 512],
                            start=(dc == 0),
                            stop=(dc == 7),
                        )
                    nc.vector.tensor_scalar_add(
                        out=qk_all[:, col, qb * 512 : (qb + 1) * 512],
                        in0=ps[:],
                        scalar1=bqk_sb[:, col : col + 1],
                    )

                def proj_v(kt):
                    ps = mm1p.tile([P, 256], f32, tag="scT")
                    for dc in range(8):
                        nc.tensor.matmul(
                            ps[:],
                            lhsT=xT_sb[:, dc, kt * P : (kt + 1) * P],
                            rhs=wv_sb[:, dc, :],
                            start=(dc == 0),
                            stop=(dc == 7),
                        )
                    nc.vector.tensor_tensor(v_sb[:, kt, :], ps[:], bvr_sb[:], Alu.add)

                def passA_strip(qh, pr, ks):
                    pss = [
                        mm1p.tile([P, 512], f32, tag="scT", name=f"ps{i}")
                        for i in range(4)
                    ]
                    eo = []
                    for q2 in range(2):
                        qs = slice(qh * QH + q2 * 512, qh * QH + (q2 + 1) * 512)
                        nc.tensor.matmul(
                            pss[2 * q2][:],
                            lhsT=qk_all[0:64, 2 * pr + 1, ks * P : (ks + 1) * P],
                            rhs=qk_all[0:64, 2 * pr, qs],
                            tile_position=(0, 0),
                            start=True,
                            stop=True,
                        )
                        nc.tensor.matmul(
                            pss[2 * q2 + 1][:],
                            lhsT=qk_all[64:128, 2 * pr + 1, ks * P : (ks + 1) * P],
                            rhs=qk_all[64:128, 2 * pr, qs],
                            tile_position=(64, 0),
                            start=True,
                            stop=True,
                        )
                    eA = ep.tile([P, QH], bf16, tag="e")
                    eB = ep.tile([P, QH], bf16, tag="e")
                    for q2 in range(2):
                        os_ = slice(q2 * 512, (q2 + 1) * 512)
                        nc.scalar.activation(
                            eA[:, os_], pss[2 * q2][:], Act.Exp, scale=0.125
                        )
                        nc.scalar.activation(
                            eB[:, os_], pss[2 * q2 + 1][:], Act.Exp, scale=0.125
                        )
                    return eA, eB

                def passB_strip(qh, pr, ks, eA, eB, po, prs):
                    h0, h1 = 2 * pr, 2 * pr + 1
                    ct = ctp.tile([P, QH], bf16, tag="ct")
                    nc.sync.dma_start(
                        out=ct[:],
                        in_=ct_d[ks * P : (ks + 1) * P, qh * QH : (qh + 1) * QH],
                    )
                    aA = app.tile([P, QH], bf16, tag="a")
                    nc.vector.tensor_tensor(aA[:], eA[:], ct[:], Alu.mult)
                    aB = app.tile([P, QH], bf16, tag="a")
                    nc.vector.tensor_tensor(aB[:], eB[:], ct[:], Alu.mult)
                    st, sp = (ks == 0), (ks == NK - 1)
                    for q2 in range(2):
                        os_ = slice(q2 * 512, (q2 + 1) * 512)
                        nc.tensor.matmul(
                            prs[0:64, os_],
                            lhsT=mk_sb[:, ks, :],
                            rhs=eA[:, os_],
                            tile_position=(0, 0),
                            start=st,
                            stop=sp,
                        )
                        nc.tensor.matmul(
                            prs[64:128, os_],
                            lhsT=mk_sb[:, ks, :],
                            rhs=eB[:, os_],
                            tile_position=(0, 64),
                            start=st,
                            stop=sp,
                        )
                        nc.tensor.matmul(
                            po[0:64, os_],
                            lhsT=v_sb[:, ks, h0 * 64 : (h0 + 1) * 64],
                            rhs=aA[:, os_],
                            tile_position=(0, 0),
                            start=st,
                            stop=sp,
                        )
                        nc.tensor.matmul(
                            po[64:128, os_],
                            lhsT=v_sb[:, ks, h1 * 64 : (h1 + 1) * 64],
                            rhs=aB[:, os_],
                            tile_position=(0, 64),
                            start=st,
                            stop=sp,
                        )

                def finishB(qh, pr, po, prs):
                    rcs = smallp.tile([P, QH], f32, tag="rcs")
                    scr = smallp.tile([P, QH], f32, tag="scr")
                    nc.vector.reciprocal_approx_accurate(rcs[:], prs[:], scr[:])
                    ob = smallp.tile([P, QH], f32, tag="ob")
                    nc.vector.tensor_tensor(ob[:], po[:], rcs[:], Alu.mult)
                    nc.sync.dma_start(
                        out=out_d[pr * P : (pr + 1) * P, qh * QH : (qh + 1) * QH],
                        in_=ob[:],
                    )

                phases = [(qh, pr) for qh in range(2) for pr in range(2)]
                # pair-0 Q/K up front; pair-1 Q/K + all of V are deferred into
                # phase-0 iterations (no pass B there yet).
                for qb in range(4):
                    proj_qk(0, qb)
                    proj_qk(1, qb)

                es_all = {}
                bstate = {}
                NITER = NPH * NK + BOFF
                for g in range(NITER):
                    ph, ks = g // NK, g % NK
                    if g < NPH * NK:
                        qh, pr = phases[ph]
                        es_all.setdefault(ph, []).append(passA_strip(qh, pr, ks))
                        if ph == 0:
                            proj_v(ks)
                            if ks < 8:
                                proj_qk(2 + ks // 4, ks % 4)
                    bg = g - BOFF
                    if bg >= 0:
                        bph, bks = bg // NK, bg % NK
                        bqh, bpr = phases[bph]
                        if bks == 0:
                            b_po = pop.tile([P, QH], f32, tag="po", name="po")
                            b_prs = prsp.tile(
                                [P, QH], f32, tag="prs", name="prs"
                            )
                            bstate[bph] = (b_po, b_prs)
                        b_po, b_prs = bstate[bph]
                        passB_strip(bqh, bpr, bks, *es_all[bph][bks], b_po, b_prs)
                        if bks == NK - 1:
                            finishB(bqh, bpr, b_po, b_prs)
                            del bstate[bph]
                            del es_all[bph]

    nc.finalize()
    return nc


def _get_program():
    if "nc" not in _prog_cache:
        _prog_cache["nc"] = _build_program()
    return _prog_cache["nc"]


def kernel(x, attention_mask, C_prior, Wq, bq, Wk, bk, Wv, bv):
    from concourse.bass_utils import run_bass_kernel_spmd

    x = np.asarray(x, dtype=np.float32)
    attention_mask = np.asarray(attention_mask)
    C_prior = np.asarray(C_prior, dtype=np.float32)
    Wq = np.asarray(Wq, dtype=np.float32)
    Wk = np.asarray(Wk, dtype=np.float32)
    Wv = np.asarray(Wv, dtype=np.float32)
    bq = np.asarray(bq, dtype=np.float32)
    bk = np.asarray(bk, dtype=np.float32)
    bv = np.asarray(bv, dtype=np.float32)
    bf = ml_dtypes.bfloat16

    WqT, WkT, WvT = Wq.T, Wk.T, Wv.T  # [in D, out D]
    maskf = attention_mask.astype(np.float32)  # [B, S]

    in_maps = []
    for c in range(NCORES):
        b, hg = c // 4, c % 4
        heads = [4 * hg + i for i in range(HEADS_PER_CORE)]
        xT = np.ascontiguousarray(x[b].T).astype(bf)  # [D, S]

        wqk = np.empty((D, 512), np.float32)
        bqk = np.zeros((P, 4), np.float32)
        for pr in range(2):
            h0, h1 = heads[2 * pr], heads[2 * pr + 1]
            wqk[:, (2 * pr) * P : (2 * pr) * P + 64] = WqT[:, h0 * 64 : h0 * 64 + 64]
            wqk[:, (2 * pr) * P + 64 : (2 * pr + 1) * P] = WqT[
                :, h1 * 64 : h1 * 64 + 64
            ]
            wqk[:, (2 * pr + 1) * P : (2 * pr + 1) * P + 64] = WkT[
                :, h0 * 64 : h0 * 64 + 64
            ]
            wqk[:, (2 * pr + 1) * P + 64 : (2 * pr + 2) * P] = WkT[
                :, h1 * 64 : h1 * 64 + 64
            ]
            bqk[0:64, 2 * pr] = bq[h0 * 64 : h0 * 64 + 64]
            bqk[64:128, 2 * pr] = bq[h1 * 64 : h1 * 64 + 64]
            bqk[0:64, 2 * pr + 1] = bk[h0 * 64 : h0 * 64 + 64]
            bqk[64:128, 2 * pr + 1] = bk[h1 * 64 : h1 * 64 + 64]

        wv = np.ascontiguousarray(WvT[:, heads[0] * 64 : (heads[-1] + 1) * 64]).astype(bf)
        bvr = np.ascontiguousarray(
            np.broadcast_to(
                bv[heads[0] * 64 : (heads[-1] + 1) * 64][None, :], (P, 256)
            )
        )
        m = maskf[b]  # [S]
        ct = (C_prior[b].T * m[:, None]).astype(bf)  # [S(k), S(q)] * mask[k]
        mkcol = m.reshape(S // P, P).T.astype(bf)  # [P, 16]
        mk = np.ascontiguousarray(
            np.repeat(mkcol[:, :, None], 64, axis=2).reshape(P, -1)
        )  # [P, 16*64]

        in_maps.append(
            {
                "xT": xT,
                "wqk": wqk.astype(bf),
                "wv": wv,
                "bqk": bqk,
                "bvr": bvr,
                "ct": ct,
                "mk": mk,
            }
        )

    nc = _get_program()
    trace = bool(int(os.environ.get("BASS_KERNEL_TRACE", "0")))
    res = run_bass_kernel_spmd(nc, in_maps, list(range(NCORES)), trace=trace)
    if trace:
        print(f"HW exec time: {res.exec_time_ns} ns")
        _prog_cache["last_exec_time_ns"] = res.exec_time_ns
        _prog_cache["last_trace"] = res.instructions_and_trace

    out = np.empty((B, S, D), np.float32)
    for c in range(NCORES):
        b, hg = c // 4, c % 4
        co = res.results[c]["out"]  # [256, S]
        for i in range(HEADS_PER_CORE):
            h = 4 * hg + i
            out[b, :, h * 64 : (h + 1) * 64] = co[i * 64 : (i + 1) * 64, :].T
    return out


# revision 20
# speedup vs baseline: 1.0815x; 1.0053x over previous
"""Trainium2 Bass kernel for BertSelfAttention with C_prior multiply.

Reference (per batch b):
  q/k/v = x @ W{q,k,v}.T + b{q,k,v}            -> [S, D], split into H=16 heads of W=64
  scores = q k^T / sqrt(W); mask; softmax over k
  attn = softmax(scores) * C_prior[b]
  out = attn @ v                               -> [B, S, D]

Shapes: B=2, S=2048, D=1024, H=16, W=64.

Sharding: 8 cores; core c owns batch b=c//4 and 4 consecutive heads
(hg=c%4 -> heads 4*hg..4*hg+3). The whole per-(b,h) score block stays local.

Device layout strategy (per core):
  - Host pre-transposes inputs so the device never transposes anything big:
      xT = x[b].T                                  [D, S]
      wqk = per-pair [Wq_h0^T|Wq_h1^T|Wk_h0^T|Wk_h1^T] column blocks
      ct  = (C_prior[b].T * mask) as bf16          [S, S]   (k-major)
  - Projections produce Q^T/K^T in [w, q] layout and V in natural [k, w].
  - scoresT = K Q^T computed directly in [k, q] layout (matmul lhsT=K^T,
    rhs=Q^T), so softmax's exp output feeds the A@V matmul with k already
    on partitions -- no on-chip transpose of the big attention matrix.
  - softmax denominator: ones(mask)-vector matmul over expS^T partitions,
    accumulated in PSUM across k-strips; exp skips max-subtraction
    (scores ~ N(0,1), no overflow risk in fp32).
  - attn*C: single VE bf16 tensor_tensor multiply per strip.
  - Output O^T [w, q] is written per head; the host transposes the small
    result during the gather/unshard step.

Matmuls run as float32r (full PE rate) for the fp32 path; the attention
matrix path (expS^T, C^T, V) runs bf16.
"""

import os

import numpy as np
import ml_dtypes

B, S, D, H, W = 2, 2048, 1024, 16, 64
NCORES = 8
HEADS_PER_CORE = 4
P = 128
QH = S // 2  # q processed in two halves of 1024 to fit PSUM

_prog_cache = {}


def _build_program():
    import concourse.mybir as mybir
    import concourse.tile as tile
    from concourse import bacc

    dt = mybir.dt
    f32, bf16 = dt.float32, dt.bfloat16
    Alu = mybir.AluOpType
    Act = mybir.ActivationFunctionType

    nc = bacc.Bacc("TRN2", target_bir_lowering=False)

    xT_d = nc.declare_dram_parameter("xT", [D, S], bf16, isOutput=False)
    wqk_d = nc.declare_dram_parameter("wqk", [D, 512], bf16, isOutput=False)
    wv_d = nc.declare_dram_parameter("wv", [D, 256], bf16, isOutput=False)
    bqk_d = nc.declare_dram_parameter("bqk", [P, 4], f32, isOutput=False)
    bvr_d = nc.declare_dram_parameter("bvr", [P, 256], f32, isOutput=False)
    ct_d = nc.declare_dram_parameter("ct", [S, S], bf16, isOutput=False)
    mk_d = nc.declare_dram_parameter("mk", [P, 16 * 64], bf16, isOutput=False)
    out_d = nc.declare_dram_parameter("out", [256, S], f32, isOutput=True)

    NK = S // P  # 16 k-strips
    NPH = 4  # phases: (qh, pair)
    BOFF = 4  # pass B trails pass A by 4 strips

    with tile.TileContext(nc) as tc:
        with tc.tile_pool(name="persist", bufs=1) as persist:
            qk_all = persist.tile([P, 4, S], bf16)
            v_sb = persist.tile([P, NK, 256], bf16)
            bqk_sb = persist.tile([P, 4], f32)
            bvr_sb = persist.tile([P, 256], f32)
            mk_sb = persist.tile([P, NK, 64], bf16)
            xT_sb = persist.tile([P, 8, S], bf16)
            wqk_sb = persist.tile([P, 8, 512], bf16)
            wv_sb = persist.tile([P, 8, 256], bf16)
            nc.sync.dma_start(out=bqk_sb[:], in_=bqk_d[:])
            nc.sync.dma_start(out=bvr_sb[:], in_=bvr_d[:])
            nc.sync.dma_start(out=mk_sb[:], in_=mk_d[:])
            xT_r = xT_d.rearrange("(o p) q -> p o q", p=P)
            wqk_r = wqk_d.rearrange("(o p) m -> p o m", p=P)
            wv_r = wv_d.rearrange("(o p) m -> p o m", p=P)
            for dc in range(8):
                nc.sync.dma_start(out=wqk_sb[:, dc, :], in_=wqk_r[:, dc, :])
                nc.sync.dma_start(out=xT_sb[:, dc, :], in_=xT_r[:, dc, :])
                nc.sync.dma_start(out=wv_sb[:, dc, :], in_=wv_r[:, dc, :])

            with tc.tile_pool(name="estr", bufs=14) as ep, tc.tile_pool(name="astr", bufs=6) as app, tc.tile_pool(
                name="ctp", bufs=6
            ) as ctp, tc.tile_pool(name="small", bufs=1) as smallp, tc.tile_pool(
                name="mm1ps", bufs=4, space="PSUM"
            ) as mm1p, tc.tile_pool(
                name="ops", bufs=1, space="PSUM"
            ) as pop, tc.tile_pool(
                name="rsps", bufs=1, space="PSUM"
            ) as prsp:

                def proj_qk(col, qb):
                    ps = mm1p.tile([P, 512], f32, tag="scT")
                    for dc in range(8):
                        nc.tensor.matmul(
                            ps[:],
                            lhsT=wqk_sb[:, dc, col * P : (col + 1) * P],
                            rhs=xT_sb[:, dc, qb * 512 : (qb + 1) * 512],
                            start=(dc == 0),
                            stop=(dc == 7),
                        )
                    nc.vector.tensor_scalar_add(
                        out=qk_all[:, col, qb * 512 : (qb + 1) * 512],
                        in0=ps[:],
                        scalar1=bqk_sb[:, col : col + 1],
                    )

                def proj_v(kt):
                    ps = mm1p.tile([P, 256], f32, tag="scT")
                    for dc in range(8):
                        nc.tensor.matmul(
                            ps[:],
                            lhsT=xT_sb[:, dc, kt * P : (kt + 1) * P],
                            rhs=wv_sb[:, dc, :],
                            start=(dc == 0),
                            stop=(dc == 7),
                        )
                    nc.vector.tensor_tensor(v_sb[:, kt, :], ps[:], bvr_sb[:], Alu.add)

                def passA_strip(qh, pr, ks):
                    pss = [
                        mm1p.tile([P, 512], f32, tag="scT", name=f"ps{i}")
                        for i in range(4)
                    ]
                    eo = []
                    for q2 in range(2):
                        qs = slice(qh * QH + q2 * 512, qh * QH + (q2 + 1) * 512)
                        nc.tensor.matmul(
                            pss[2 * q2][:],
                            lhsT=qk_all[0:64, 2 * pr + 1, ks * P : (ks + 1) * P],
                            rhs=qk_all[0:64, 2 * pr, qs],
                            tile_position=(0, 0),
                            start=True,
                            stop=True,
                        )
                        nc.tensor.matmul(
                            pss[2 * q2 + 1][:],
                            lhsT=qk_all[64:128, 2 * pr + 1, ks * P : (ks + 1) * P],
                            rhs=qk_all[64:128, 2 * pr, qs],
                            tile_position=(64, 0),
                            start=True,
                            stop=True,
                        )
                    eA = ep.tile([P, QH], bf16, tag="e")
                    eB = ep.tile([P, QH], bf16, tag="e")
                    for q2 in range(2):
                        os_ = slice(q2 * 512, (q2 + 1) * 512)
                        nc.scalar.activation(
                            eA[:, os_], pss[2 * q2][:], Act.Exp, scale=0.125
                        )
                        nc.scalar.activation(
                            eB[:, os_], pss[2 * q2 + 1][:], Act.Exp, scale=0.125
                        )
                    return eA, eB

                def passB_strip(qh, pr, ks, eA, eB, po, prs):
                    h0, h1 = 2 * pr, 2 * pr + 1
                    ct = ctp.tile([P, QH], bf16, tag="ct")
                    nc.sync.dma_start(
                        out=ct[:],
                        in_=ct_d[ks * P : (ks + 1) * P, qh * QH : (qh + 1) * QH],
                    )
                    aA = app.tile([P, QH], bf16, tag="a")
                    nc.vector.tensor_tensor(aA[:], eA[:], ct[:], Alu.mult)
                    aB = app.tile([P, QH], bf16, tag="a")
                    nc.vector.tensor_tensor(aB[:], eB[:], ct[:], Alu.mult)
                    st, sp = (ks == 0), (ks == NK - 1)
                    for q2 in range(2):
                        os_ = slice(q2 * 512, (q2 + 1) * 512)
                        nc.tensor.matmul(
                            prs[0:64, os_],
                            lhsT=mk_sb[:, ks, :],
                            rhs=eA[:, os_],
                            tile_position=(0, 0),
                            start=st,
                            stop=sp,
                        )
                        nc.tensor.matmul(
                            prs[64:128, os_],
                            lhsT=mk_sb[:, ks, :],
                            rhs=eB[:, os_],
                            tile_position=(0, 64),
                            start=st,
                            stop=sp,
                        )
                        nc.tensor.matmul(
                            po[0:64, os_],
                            lhsT=v_sb[:, ks, h0 * 64 : (h0 + 1) * 64],
                            rhs=aA[:, os_],
                            tile_position=(0, 0),
                            start=st,
                            stop=sp,
                        )
                        nc.tensor.matmul(
                            po[64:128, os_],
                            lhsT=v_sb[:, ks, h1 * 64 : (h1 + 1) * 64],
                            rhs=aB[:, os_],
                            tile_position=(0, 64),
                            start=st,
                            stop=sp,
                        )

                def finishB(qh, pr, po, prs):
                    rcs = smallp.tile([P, QH], f32, tag="rcs")
                    scr = smallp.tile([P, QH], f32, tag="scr")
                    nc.vector.reciprocal_approx_accurate(rcs[:], prs[:], scr[:])
                    ob = smallp.tile([P, QH], f32, tag="ob")
                    nc.vector.tensor_tensor(ob[:], po[:], rcs[:], Alu.mult)
                    nc.sync.dma_start(
                        out=out_d[pr * P : (pr + 1) * P, qh * QH : (qh + 1) * QH],
                        in_=ob[:],
                    )

                phases = [(qh, pr) for qh in range(2) for pr in range(2)]
                # pair-0 Q/K up front; pair-1 Q/K + all of V are deferred into
                # phase-0 iterations (no pass B there yet).
                for qb in range(4):
                    proj_qk(0, qb)
                    proj_qk(1, qb)

                es_all = {}
                bstate = {}
                NITER = NPH * NK + BOFF
                for g in range(NITER):
                    ph, ks = g // NK, g % NK
                    if g < NPH * NK:
                        qh, pr = phases[ph]
                        es_all.setdefault(ph, []).append(passA_strip(qh, pr, ks))
                        if ph == 0:
                            proj_v(ks)
                            if ks < 8:
                                proj_qk(2 + ks // 4, ks % 4)
                    bg = g - BOFF
                    if bg >= 0:
                        bph, bks = bg // NK, bg % NK
                        bqh, bpr = phases[bph]
                        if bks == 0:
                            b_po = pop.tile([P, QH], f32, tag="po", name="po")
                            b_prs = prsp.tile(
                                [P, QH], f32, tag="prs", name="prs"
                            )
                            bstate[bph] = (b_po, b_prs)
                        b_po, b_prs = bstate[bph]
                        passB_strip(bqh, bpr, bks, *es_all[bph][bks], b_po, b_prs)
                        if bks == NK - 1:
                            finishB(bqh, bpr, b_po, b_prs)
                            del bstate[bph]
                            del es_all[bph]

    nc.finalize()
    return nc


def _get_program():
    if "nc" not in _prog_cache:
        _prog_cache["nc"] = _build_program()
    return _prog_cache["nc"]


def kernel(x, attention_mask, C_prior, Wq, bq, Wk, bk, Wv, bv):
    from concourse.bass_utils import run_bass_kernel_spmd

    x = np.asarray(x, dtype=np.float32)
    attention_mask = np.asarray(attention_mask)
    C_prior = np.asarray(C_prior, dtype=np.float32)
    Wq = np.asarray(Wq, dtype=np.float32)
    Wk = np.asarray(Wk, dtype=np.float32)
    Wv = np.asarray(Wv, dtype=np.float32)
    bq = np.asarray(bq, dtype=np.float32)
    bk = np.asarray(bk, dtype=np.float32)
    bv = np.asarray(bv, dtype=np.float32)
    bf = ml_dtypes.bfloat16

    WqT, WkT, WvT = Wq.T, Wk.T, Wv.T  # [in D, out D]
    maskf = attention_mask.astype(np.float32)  # [B, S]

    in_maps = []
    for c in range(NCORES):
        b, hg = c // 4, c % 4
        heads = [4 * hg + i for i in range(HEADS_PER_CORE)]
        xT = np.ascontiguousarray(x[b].T).astype(bf)  # [D, S]

        wqk = np.empty((D, 512), np.float32)
        bqk = np.zeros((P, 4), np.float32)
        for pr in range(2):
            h0, h1 = heads[2 * pr], heads[2 * pr + 1]
            wqk[:, (2 * pr) * P : (2 * pr) * P + 64] = WqT[:, h0 * 64 : h0 * 64 + 64]
            wqk[:, (2 * pr) * P + 64 : (2 * pr + 1) * P] = WqT[
                :, h1 * 64 : h1 * 64 + 64
            ]
            wqk[:, (2 * pr + 1) * P : (2 * pr + 1) * P + 64] = WkT[
                :, h0 * 64 : h0 * 64 + 64
            ]
            wqk[:, (2 * pr + 1) * P + 64 : (2 * pr + 2) * P] = WkT[
                :, h1 * 64 : h1 * 64 + 64
            ]
            bqk[0:64, 2 * pr] = bq[h0 * 64 : h0 * 64 + 64]
            bqk[64:128, 2 * pr] = bq[h1 * 64 : h1 * 64 + 64]
            bqk[0:64, 2 * pr + 1] = bk[h0 * 64 : h0 * 64 + 64]
            bqk[64:128, 2 * pr + 1] = bk[h1 * 64 : h1 * 64 + 64]

        wv = np.ascontiguousarray(WvT[:, heads[0] * 64 : (heads[-1] + 1) * 64]).astype(bf)
        bvr = np.ascontiguousarray(
            np.broadcast_to(
                bv[heads[0] * 64 : (heads[-1] + 1) * 64][None, :], (P, 256)
            )
        )
        m = maskf[b]  # [S]
        ct = (C_prior[b].T * m[:, None]).astype(bf)  # [S(k), S(q)] * mask[k]
        mkcol = m.reshape(S // P, P).T.astype(bf)  # [P, 16]
        mk = np.ascontiguousarray(
            np.repeat(mkcol[:, :, None], 64, axis=2).reshape(P, -1)
        )  # [P, 16*64]

        in_maps.append(
            {
                "xT": xT,
                "wqk": wqk.astype(bf),
                "wv": wv,
                "bqk": bqk,
                "bvr": bvr,
                "ct": ct,
                "mk": mk,
            }
        )

    nc = _get_program()
    trace = bool(int(os.environ.get("BASS_KERNEL_TRACE", "0")))
    res = run_bass_kernel_spmd(nc, in_maps, list(range(NCORES)), trace=trace)
    if trace:
        print(f"HW exec time: {res.exec_time_ns} ns")
        _prog_cache["last_exec_time_ns"] = res.exec_time_ns
        _prog_cache["last_trace"] = res.instructions_and_trace

    out = np.empty((B, S, D), np.float32)
    for c in range(NCORES):
        b, hg = c // 4, c % 4
        co = res.results[c]["out"]  # [256, S]
        for i in range(HEADS_PER_CORE):
            h = 4 * hg + i
            out[b, :, h * 64 : (h + 1) * 64] = co[i * 64 : (i + 1) * 64, :].T
    return out


# revision 22
# speedup vs baseline: 1.0905x; 1.0083x over previous
"""Trainium2 Bass kernel for BertSelfAttention with C_prior multiply.

Reference (per batch b):
  q/k/v = x @ W{q,k,v}.T + b{q,k,v}            -> [S, D], split into H=16 heads of W=64
  scores = q k^T / sqrt(W); mask; softmax over k
  attn = softmax(scores) * C_prior[b]
  out = attn @ v                               -> [B, S, D]

Shapes: B=2, S=2048, D=1024, H=16, W=64.

Sharding: 8 cores; core c owns batch b=c//4 and 4 consecutive heads
(hg=c%4 -> heads 4*hg..4*hg+3). The whole per-(b,h) score block stays local.

Device layout strategy (per core):
  - Host pre-transposes inputs so the device never transposes anything big:
      xT = x[b].T                                  [D, S]
      wqk = per-pair [Wq_h0^T|Wq_h1^T|Wk_h0^T|Wk_h1^T] column blocks
      ct  = (C_prior[b].T * mask) as bf16          [S, S]   (k-major)
  - Projections produce Q^T/K^T in [w, q] layout and V in natural [k, w].
  - scoresT = K Q^T computed directly in [k, q] layout (matmul lhsT=K^T,
    rhs=Q^T), so softmax's exp output feeds the A@V matmul with k already
    on partitions -- no on-chip transpose of the big attention matrix.
  - softmax denominator: ones(mask)-vector matmul over expS^T partitions,
    accumulated in PSUM across k-strips; exp skips max-subtraction
    (scores ~ N(0,1), no overflow risk in fp32).
  - attn*C: single VE bf16 tensor_tensor multiply per strip.
  - Output O^T [w, q] is written per head; the host transposes the small
    result during the gather/unshard step.

Matmuls run as float32r (full PE rate) for the fp32 path; the attention
matrix path (expS^T, C^T, V) runs bf16.
"""

import os

import numpy as np
import ml_dtypes

B, S, D, H, W = 2, 2048, 1024, 16, 64
NCORES = 8
HEADS_PER_CORE = 4
P = 128
QH = S // 2  # q processed in two halves of 1024 to fit PSUM

_prog_cache = {}


def _build_program():
    import concourse.mybir as mybir
    import concourse.tile as tile
    from concourse import bacc

    dt = mybir.dt
    f32, bf16 = dt.float32, dt.bfloat16
    Alu = mybir.AluOpType
    Act = mybir.ActivationFunctionType

    nc = bacc.Bacc("TRN2", target_bir_lowering=False)

    xT_d = nc.declare_dram_parameter("xT", [D, S], bf16, isOutput=False)
    wqk_d = nc.declare_dram_parameter("wqk", [D, 512], bf16, isOutput=False)
    wv_d = nc.declare_dram_parameter("wv", [D, 256], bf16, isOutput=False)
    bqk_d = nc.declare_dram_parameter("bqk", [P, 4], f32, isOutput=False)
    sel_d = nc.declare_dram_parameter("sel", [P, 2], f32, isOutput=False)
    bvr_d = nc.declare_dram_parameter("bvr", [P, 256], f32, isOutput=False)
    ct_d = nc.declare_dram_parameter("ct", [S, S], bf16, isOutput=False)
    mk_d = nc.declare_dram_parameter("mk", [P, 16 * 64], bf16, isOutput=False)
    out_d = nc.declare_dram_parameter("out", [256, S], f32, isOutput=True)

    NK = S // P  # 16 k-strips
    NPH = 4  # phases: (qh, pair)
    BOFF = 4  # pass B trails pass A by 4 strips

    with tile.TileContext(nc) as tc:
        with tc.tile_pool(name="persist", bufs=1) as persist:
            q_all = persist.tile([P, 2, S], bf16)
            kpad = persist.tile([P, 4, S], bf16)
            sel_sb = persist.tile([P, 2], f32)
            v_sb = persist.tile([P, NK, 256], bf16)
            bqk_sb = persist.tile([P, 4], f32)
            bvr_sb = persist.tile([P, 256], f32)
            mk_sb = persist.tile([P, NK, 64], bf16)
            xT_sb = persist.tile([P, 8, S], bf16)
            wqk_sb = persist.tile([P, 8, 512], bf16)
            wv_sb = persist.tile([P, 8, 256], bf16)
            nc.sync.dma_start(out=bqk_sb[:], in_=bqk_d[:])
            nc.sync.dma_start(out=sel_sb[:], in_=sel_d[:])
            nc.sync.dma_start(out=bvr_sb[:], in_=bvr_d[:])
            nc.sync.dma_start(out=mk_sb[:], in_=mk_d[:])
            xT_r = xT_d.rearrange("(o p) q -> p o q", p=P)
            wqk_r = wqk_d.rearrange("(o p) m -> p o m", p=P)
            wv_r = wv_d.rearrange("(o p) m -> p o m", p=P)
            for dc in range(8):
                nc.sync.dma_start(out=wqk_sb[:, dc, :], in_=wqk_r[:, dc, :])
                nc.sync.dma_start(out=xT_sb[:, dc, :], in_=xT_r[:, dc, :])
                nc.sync.dma_start(out=wv_sb[:, dc, :], in_=wv_r[:, dc, :])

            with tc.tile_pool(name="estr", bufs=14) as ep, tc.tile_pool(name="astr", bufs=6) as app, tc.tile_pool(
                name="ctp", bufs=6
            ) as ctp, tc.tile_pool(name="small", bufs=1) as smallp, tc.tile_pool(
                name="mm1ps", bufs=4, space="PSUM"
            ) as mm1p, tc.tile_pool(
                name="ops", bufs=1, space="PSUM"
            ) as pop, tc.tile_pool(
                name="rsps", bufs=1, space="PSUM"
            ) as prsp:

                def proj_qk(col, qb):
                    pr_, isk = col // 2, col % 2
                    ps = mm1p.tile([P, 512], f32, tag="scT")
                    for dc in range(8):
                        nc.tensor.matmul(
                            ps[:],
                            lhsT=wqk_sb[:, dc, col * P : (col + 1) * P],
                            rhs=xT_sb[:, dc, qb * 512 : (qb + 1) * 512],
                            start=(dc == 0),
                            stop=(dc == 7),
                        )
                    qbs = slice(qb * 512, (qb + 1) * 512)
                    if isk == 0:
                        nc.vector.tensor_scalar_add(
                            out=q_all[:, pr_, qbs],
                            in0=ps[:],
                            scalar1=bqk_sb[:, col : col + 1],
                        )
                    else:
                        nc.vector.tensor_scalar(
                            out=kpad[:, 2 * pr_, qbs],
                            in0=ps[:],
                            scalar1=bqk_sb[:, col : col + 1],
                            scalar2=sel_sb[:, 0:1],
                            op0=Alu.add,
                            op1=Alu.mult,
                        )
                        nc.vector.tensor_scalar(
                            out=kpad[:, 2 * pr_ + 1, qbs],
                            in0=ps[:],
                            scalar1=bqk_sb[:, col : col + 1],
                            scalar2=sel_sb[:, 1:2],
                            op0=Alu.add,
                            op1=Alu.mult,
                        )

                def proj_v(kt):
                    ps = mm1p.tile([P, 256], f32, tag="scT")
                    for dc in range(8):
                        nc.tensor.matmul(
                            ps[:],
                            lhsT=xT_sb[:, dc, kt * P : (kt + 1) * P],
                            rhs=wv_sb[:, dc, :],
                            start=(dc == 0),
                            stop=(dc == 7),
                        )
                    nc.vector.tensor_tensor(v_sb[:, kt, :], ps[:], bvr_sb[:], Alu.add)

                def passA_strip(qh, pr, ks):
                    pss = [
                        mm1p.tile([P, 512], f32, tag="scT", name=f"ps{i}")
                        for i in range(4)
                    ]
                    for q2 in range(2):
                        qs = slice(qh * QH + q2 * 512, qh * QH + (q2 + 1) * 512)
                        rhsq = q_all[:, pr, qs]
                        nc.tensor.matmul(
                            pss[2 * q2][0:64, :],
                            lhsT=kpad[:, 2 * pr, ks * P : ks * P + 64],
                            rhs=rhsq,
                            tile_position=(0, 0),
                            start=True,
                            stop=True,
                        )
                        nc.tensor.matmul(
                            pss[2 * q2][64:128, :],
                            lhsT=kpad[:, 2 * pr, ks * P + 64 : (ks + 1) * P],
                            rhs=rhsq,
                            tile_position=(0, 64),
                            start=True,
                            stop=True,
                        )
                        nc.tensor.matmul(
                            pss[2 * q2 + 1][0:64, :],
                            lhsT=kpad[:, 2 * pr + 1, ks * P : ks * P + 64],
                            rhs=rhsq,
                            tile_position=(0, 0),
                            start=True,
                            stop=True,
                        )
                        nc.tensor.matmul(
                            pss[2 * q2 + 1][64:128, :],
                            lhsT=kpad[:, 2 * pr + 1, ks * P + 64 : (ks + 1) * P],
                            rhs=rhsq,
                            tile_position=(0, 64),
                            start=True,
                            stop=True,
                        )
                    eA = ep.tile([P, QH], bf16, tag="e")
                    eB = ep.tile([P, QH], bf16, tag="e")
                    for q2 in range(2):
                        os_ = slice(q2 * 512, (q2 + 1) * 512)
                        nc.scalar.activation(
                            eA[:, os_], pss[2 * q2][:], Act.Exp, scale=0.125
                        )
                        nc.scalar.activation(
                            eB[:, os_], pss[2 * q2 + 1][:], Act.Exp, scale=0.125
                        )
                    return eA, eB

                def passB_strip(qh, pr, ks, eA, eB, po, prs):
                    h0, h1 = 2 * pr, 2 * pr + 1
                    ct = ctp.tile([P, QH], bf16, tag="ct")
                    nc.sync.dma_start(
                        out=ct[:],
                        in_=ct_d[ks * P : (ks + 1) * P, qh * QH : (qh + 1) * QH],
                    )
                    aA = app.tile([P, QH], bf16, tag="a")
                    nc.vector.tensor_tensor(aA[:], eA[:], ct[:], Alu.mult)
                    aB = app.tile([P, QH], bf16, tag="a")
                    nc.vector.tensor_tensor(aB[:], eB[:], ct[:], Alu.mult)
                    st, sp = (ks == 0), (ks == NK - 1)
                    for q2 in range(2):
                        os_ = slice(q2 * 512, (q2 + 1) * 512)
                        nc.tensor.matmul(
                            prs[0:64, os_],
                            lhsT=mk_sb[:, ks, :],
                            rhs=eA[:, os_],
                            tile_position=(0, 0),
                            start=st,
                            stop=sp,
                        )
                        nc.tensor.matmul(
                            prs[64:128, os_],
                            lhsT=mk_sb[:, ks, :],
                            rhs=eB[:, os_],
                            tile_position=(0, 64),
                            start=st,
                            stop=sp,
                        )
                        nc.tensor.matmul(
                            po[0:64, os_],
                            lhsT=v_sb[:, ks, h0 * 64 : (h0 + 1) * 64],
                            rhs=aA[:, os_],
                            tile_position=(0, 0),
                            start=st,
                            stop=sp,
                        )
                        nc.tensor.matmul(
                            po[64:128, os_],
                            lhsT=v_sb[:, ks, h1 * 64 : (h1 + 1) * 64],
                            rhs=aB[:, os_],
                            tile_position=(0, 64),
                            start=st,
                            stop=sp,
                        )

                def finishB(qh, pr, po, prs):
                    rcs = smallp.tile([P, QH], f32, tag="rcs")
                    scr = smallp.tile([P, QH], f32, tag="scr")
                    nc.vector.reciprocal_approx_accurate(rcs[:], prs[:], scr[:])
                    ob = smallp.tile([P, QH], f32, tag="ob")
                    nc.vector.tensor_tensor(ob[:], po[:], rcs[:], Alu.mult)
                    nc.sync.dma_start(
                        out=out_d[pr * P : (pr + 1) * P, qh * QH : (qh + 1) * QH],
                        in_=ob[:],
                    )

                phases = [(qh, pr) for qh in range(2) for pr in range(2)]
                # pair-0 Q/K up front; pair-1 Q/K + all of V are deferred into
                # phase-0 iterations (no pass B there yet).
                for qb in range(4):
                    proj_qk(0, qb)
                    proj_qk(1, qb)

                es_all = {}
                bstate = {}
                NITER = NPH * NK + BOFF
                for g in range(NITER):
                    ph, ks = g // NK, g % NK
                    if g < NPH * NK:
                        qh, pr = phases[ph]
                        es_all.setdefault(ph, []).append(passA_strip(qh, pr, ks))
                        if ph == 0:
                            proj_v(ks)
                            if ks < 8:
                                proj_qk(2 + ks // 4, ks % 4)
                    bg = g - BOFF
                    if bg >= 0:
                        bph, bks = bg // NK, bg % NK
                        bqh, bpr = phases[bph]
                        if bks == 0:
                            b_po = pop.tile([P, QH], f32, tag="po", name="po")
                            b_prs = prsp.tile(
                                [P, QH], f32, tag="prs", name="prs"
                            )
                            bstate[bph] = (b_po, b_prs)
                        b_po, b_prs = bstate[bph]
                        passB_strip(bqh, bpr, bks, *es_all[bph][bks], b_po, b_prs)
                        if bks == NK - 1:
                            finishB(bqh, bpr, b_po, b_prs)
                            del bstate[bph]
                            del es_all[bph]

    nc.finalize()
    return nc


def _get_program():
    if "nc" not in _prog_cache:
        _prog_cache["nc"] = _build_program()
    return _prog_cache["nc"]


def kernel(x, attention_mask, C_prior, Wq, bq, Wk, bk, Wv, bv):
    from concourse.bass_utils import run_bass_kernel_spmd

    x = np.asarray(x, dtype=np.float32)
    attention_mask = np.asarray(attention_mask)
    C_prior = np.asarray(C_prior, dtype=np.float32)
    Wq = np.asarray(Wq, dtype=np.float32)
    Wk = np.asarray(Wk, dtype=np.float32)
    Wv = np.asarray(Wv, dtype=np.float32)
    bq = np.asarray(bq, dtype=np.float32)
    bk = np.asarray(bk, dtype=np.float32)
    bv = np.asarray(bv, dtype=np.float32)
    bf = ml_dtypes.bfloat16

    WqT, WkT, WvT = Wq.T, Wk.T, Wv.T  # [in D, out D]
    maskf = attention_mask.astype(np.float32)  # [B, S]

    in_maps = []
    for c in range(NCORES):
        b, hg = c // 4, c % 4
        heads = [4 * hg + i for i in range(HEADS_PER_CORE)]
        xT = np.ascontiguousarray(x[b].T).astype(bf)  # [D, S]

        wqk = np.empty((D, 512), np.float32)
        bqk = np.zeros((P, 4), np.float32)
        for pr in range(2):
            h0, h1 = heads[2 * pr], heads[2 * pr + 1]
            wqk[:, (2 * pr) * P : (2 * pr) * P + 64] = WqT[:, h0 * 64 : h0 * 64 + 64]
            wqk[:, (2 * pr) * P + 64 : (2 * pr + 1) * P] = WqT[
                :, h1 * 64 : h1 * 64 + 64
            ]
            wqk[:, (2 * pr + 1) * P : (2 * pr + 1) * P + 64] = WkT[
                :, h0 * 64 : h0 * 64 + 64
            ]
            wqk[:, (2 * pr + 1) * P + 64 : (2 * pr + 2) * P] = WkT[
                :, h1 * 64 : h1 * 64 + 64
            ]
            bqk[0:64, 2 * pr] = bq[h0 * 64 : h0 * 64 + 64]
            bqk[64:128, 2 * pr] = bq[h1 * 64 : h1 * 64 + 64]
            bqk[0:64, 2 * pr + 1] = bk[h0 * 64 : h0 * 64 + 64]
            bqk[64:128, 2 * pr + 1] = bk[h1 * 64 : h1 * 64 + 64]

        wv = np.ascontiguousarray(WvT[:, heads[0] * 64 : (heads[-1] + 1) * 64]).astype(bf)
        bvr = np.ascontiguousarray(
            np.broadcast_to(
                bv[heads[0] * 64 : (heads[-1] + 1) * 64][None, :], (P, 256)
            )
        )
        sel = np.zeros((P, 2), np.float32)
        sel[0:64, 0] = 1.0
        sel[64:128, 1] = 1.0
        m = maskf[b]  # [S]
        ct = (C_prior[b].T * m[:, None]).astype(bf)  # [S(k), S(q)] * mask[k]
        mkcol = m.reshape(S // P, P).T.astype(bf)  # [P, 16]
        mk = np.ascontiguousarray(
            np.repeat(mkcol[:, :, None], 64, axis=2).reshape(P, -1)
        )  # [P, 16*64]

        in_maps.append(
            {
                "xT": xT,
                "wqk": wqk.astype(bf),
                "wv": wv,
                "bqk": bqk,
                "sel": sel,
                "bvr": bvr,
                "ct": ct,
                "mk": mk,
            }
        )

    nc = _get_program()
    trace = bool(int(os.environ.get("BASS_KERNEL_TRACE", "0")))
    res = run_bass_kernel_spmd(nc, in_maps, list(range(NCORES)), trace=trace)
    if trace:
        print(f"HW exec time: {res.exec_time_ns} ns")
        _prog_cache["last_exec_time_ns"] = res.exec_time_ns
        _prog_cache["last_trace"] = res.instructions_and_trace

    out = np.empty((B, S, D), np.float32)
    for c in range(NCORES):
        b, hg = c // 4, c % 4
        co = res.results[c]["out"]  # [256, S]
        for i in range(HEADS_PER_CORE):
            h = 4 * hg + i
            out[b, :, h * 64 : (h + 1) * 64] = co[i * 64 : (i + 1) * 64, :].T
    return out
